# revision 41
# baseline (speedup 1.0000x reference)
"""Trainium2 Bass kernel for nn_LinearTemporalSelfAttention (B=4,T=8192,D=512,H=8).

Sharding: 8 cores = B(4) x T-halves(2). Each core owns a (b, t-half) slab
(4096 x 512) end-to-end. Cross-core data is only the KV-state einsum
(sum over full T) and the emb projection (emb_W sharded over TE within a
pair) — folded into two pair-wise bf16 AllReduces (first half of U mid-
phase-A so it overlaps compute, second half at the end).

v3 structure:
 - LN rstd batched per 16-tile group (one Ln + one Exp each) so the
   scalar engine never thrashes activation tables; groups interleave
   with the projection loop so stats DMA/vector work overlaps PE work.
 - All transposes on the PE (matmul is_transpose) + PSUM evac.
 - QKV / out projections and the KV-state einsum run fp8(e4m3)
   DoubleRow (2 k-subtiles per pass). Weights x64 on host; exp(k),
   masked v quantized to fp8 (their errors average out over T in U).
 - Per-head softmax-q 1/S applied in ONE broadcast tensor_tensor op.
 - Phase B pass 2 works in transposed space: PE-transpose LN2-normalized
   y, then a single ACT Silu per chunk applies the stylization scale
   and shift (per-partition there) while evacuating PSUM into fp8.
 - Elementwise work spread across scalar/vector/gpsimd.
"""
import numpy as np
import ml_dtypes

B, T, D, H, TE = 4, 8192, 512, 8, 2048
Dh = D // H          # 64
EPS = 1e-5
NCORES = 8
TH = T // 2          # 4096 rows per core
P = 128
NT = TH // P         # 32 row tiles
NG = 16              # tiles per stat/U group
KC = D // P          # 4 contraction chunks
TEH = TE // 2        # 1024 te rows per core
TEC = TEH // P       # 8 te chunks
CCU = 64 * H * (Dh + 1)     # 33280 floats of U_aug
CCN = CCU + 2 * D           # + emb partial
WSCALE = 64.0        # fp8 weight prescale
RWS = 1.0 / WSCALE

_CACHE: dict = {}


def _build(flags):
    has_bq, has_bk, has_bv, has_outb, has_embb = flags
    from contextlib import ExitStack
    import concourse.bass as bass
    import concourse.bacc as bacc
    import concourse.tile as tile
    import concourse.mybir as mybir
    from concourse.masks import make_identity

    f32 = mybir.dt.float32
    bf16 = mybir.dt.bfloat16
    f8 = mybir.dt.float8e4
    Alu = mybir.AluOpType
    Act = mybir.ActivationFunctionType
    DR = mybir.MatmulPerfMode.DoubleRow

    nc = bacc.Bacc("TRN2", target_bir_lowering=False, debug=False,
                   enable_asserts=True, num_devices=NCORES)

    x_in = nc.declare_dram_parameter("x", [TH, D], f32, isOutput=False)
    mk_in = nc.declare_dram_parameter("mask", [P, NT], f32, isOutput=False)
    emb_in = nc.declare_dram_parameter("embv", [P, TEC], f32, isOutput=False)
    wq_in = nc.declare_dram_parameter("wq", [P, KC, D], f8, isOutput=False)
    wk_in = nc.declare_dram_parameter("wk", [P, KC, D], f8, isOutput=False)
    wv_in = nc.declare_dram_parameter("wv", [P, KC, D], f8, isOutput=False)
    wo_in = nc.declare_dram_parameter("wo", [P, KC, D], bf16, isOutput=False)
    we_in = nc.declare_dram_parameter("we", [P, TEC, 2 * D], bf16, isOutput=False)
    vec_in = nc.declare_dram_parameter("vecs", [1, 8, D], f32, isOutput=False)
    y_out = nc.declare_dram_parameter("y", [TH, D], f32, isOutput=True)

    PAIRS = [[0, 1], [2, 3], [4, 5], [6, 7]]

    with tile.TileContext(nc) as tc, ExitStack() as ctx:
        const = ctx.enter_context(tc.tile_pool(name="const", bufs=1))
        wpool = ctx.enter_context(tc.tile_pool(name="wpool", bufs=1))
        qstash = ctx.enter_context(tc.tile_pool(name="qstash", bufs=NT))
        ystash = ctx.enter_context(tc.tile_pool(name="ystash", bufs=NT))
        stat = ctx.enter_context(tc.tile_pool(name="stat", bufs=1))
        dramp = ctx.enter_context(tc.tile_pool(name="dram", bufs=1, space="DRAM"))

        ident = const.tile([P, P], bf16)
        make_identity(nc, ident)
        eps_t = const.tile([P, 1], f32)
        nc.vector.memset(eps_t, EPS)
        ones8 = const.tile([P, H, 1], bf16)
        nc.vector.memset(ones8, 1.0)
        ones_row = const.tile([1, P], bf16)
        nc.vector.memset(ones_row, 1.0)
        one_f32 = const.tile([1, 1], f32)
        nc.vector.memset(one_f32, 1.0)

        wq_s = wpool.tile([P, KC, D], f8)
        wk_s = wpool.tile([P, KC, D], f8)
        wv_s = wpool.tile([P, KC, D], f8)
        wo_s = wpool.tile([P, KC, D], bf16)
        we_s = wpool.tile([P, TEC, 2 * D], bf16)
        mask_s = wpool.tile([P, NT], f32)
        vec_s = wpool.tile([1, 8, D], f32)

        mv_st = stat.tile([P, NT, 2], f32)
        s1_st = stat.tile([P, NT], f32)
        s2_st = stat.tile([P, NT], f32)
        rq_st = stat.tile([P, NT, H, 1], f32)
        rstd_a = stat.tile([P, NT], f32)
        nb_a = stat.tile([P, NT], f32)
        sd_a = stat.tile([P, NT], f32)
        rstd2_a = stat.tile([P, NT], f32)
        nb2_a = stat.tile([P, NT], f32)
        sd2_a = stat.tile([P, NT], f32)

        cc_in_a = dramp.tile([CCN], bf16)
        cc_out_a = dramp.tile([CCN], bf16)
        cc_in_b = dramp.tile([CCU], bf16)
        cc_out_b = dramp.tile([CCU], bf16)

        x_tiles = []
        q_tiles = []
        y_tiles = []

        with ExitStack() as ctxA:
            xstash = ctxA.enter_context(tc.tile_pool(name="xstash", bufs=NT))
            work = ctxA.enter_context(tc.tile_pool(name="work", bufs=3))
            psP = ctxA.enter_context(tc.tile_pool(name="psP", bufs=1, space="PSUM"))
            psT = ctxA.enter_context(tc.tile_pool(name="psT", bufs=2, space="PSUM"))
            psU = ctxA.enter_context(tc.tile_pool(name="psU", bufs=1, space="PSUM"))
            embp = ctxA.enter_context(tc.tile_pool(name="embp", bufs=1))

            def stats_tile(i):
                xt = xstash.tile([P, D], f32, tag="x")
                x_tiles.append(xt)
                nc.sync.dma_start(out=xt, in_=x_in[i * P:(i + 1) * P, :])
                st = work.tile([P, 6], f32, tag="st")
                nc.vector.bn_stats(out=st, in_=xt)
                nc.vector.bn_aggr(out=mv_st[:, i, :], in_=st)

            def stats_batch(g):
                sl = slice(g * 8, (g + 1) * 8)
                nc.scalar.activation(out=sd_a[:, sl], in_=mv_st[:, sl, 1],
                                     func=Act.Ln, bias=eps_t)
                nc.scalar.activation(out=rstd_a[:, sl], in_=sd_a[:, sl],
                                     func=Act.Exp, scale=-0.5)
                nc.vector.tensor_mul(out=nb_a[:, sl], in0=mv_st[:, sl, 0],
                                     in1=rstd_a[:, sl])
                nc.vector.tensor_scalar_mul(out=nb_a[:, sl], in0=nb_a[:, sl],
                                            scalar1=-1.0)

            # x loads + first stat group lead; weights follow so the first
            # projections are never queued behind bulk weight traffic
            for i in range(8):
                stats_tile(i)
            stats_batch(0)
            nc.sync.dma_start(out=wq_s, in_=wq_in[:])
            nc.sync.dma_start(out=wk_s, in_=wk_in[:])
            nc.sync.dma_start(out=wv_s, in_=wv_in[:])
            nc.sync.dma_start(out=mask_s, in_=mk_in[:])
            nc.sync.dma_start(out=vec_s, in_=vec_in[:])
            nc.sync.dma_start(out=we_s, in_=we_in[:])
            nc.sync.dma_start(out=wo_s, in_=wo_in[:])

            # ---- bias broadcast tiles (only when biases nonzero) ----
            def bcast_row(row_idx, name):
                pb = psT.tile([P, KC, P], f32, tag="pT")
                rbf = const.tile([1, D], bf16, tag="rbf_" + name)
                nc.vector.tensor_copy(out=rbf, in_=vec_s[:, row_idx, :])
                nc.tensor.matmul(out=pb[:].rearrange("p a b -> p (a b)"),
                                 lhsT=ones_row, rhs=rbf, start=True, stop=True)
                bc = const.tile([P, D], f32, tag="bc_" + name)
                nc.scalar.copy(out=bc, in_=pb[:].rearrange("p a b -> p (a b)"))
                return bc

            bq_bc = bcast_row(0, "bq") if has_bq else None
            bk_bc = bcast_row(1, "bk") if has_bk else None
            bv_bc = bcast_row(2, "bv") if has_bv else None
            ob_bc = bcast_row(3, "ob") if has_outb else None

            # ---- emb projection partial (this core's TE shard) ----
            # silu via exp-table ops only: e*sigmoid(e) = e/(1+exp(-e))
            embt = embp.tile([P, TEC], f32)
            nc.sync.dma_start(out=embt, in_=emb_in[:])
            emneg = embp.tile([P, TEC], f32)
            nc.scalar.activation(out=emneg, in_=embt, func=Act.Exp, scale=-1.0)
            nc.vector.tensor_scalar_add(out=emneg, in0=emneg, scalar1=1.0)
            nc.vector.reciprocal(out=emneg, in_=emneg)
            embs = embp.tile([P, TEC], bf16)
            nc.vector.tensor_mul(out=embs, in0=embt, in1=emneg)
            pe0 = psP.tile([P, D], f32, tag="pq", bufs=2)
            pe1 = psP.tile([P, D], f32, tag="pk")
            for j in range(TEC):
                nc.tensor.matmul(out=pe0[0:1, :], lhsT=embs[:, j:j + 1],
                                 rhs=we_s[:, j, 0:D],
                                 start=(j == 0), stop=(j == TEC - 1))
            for j in range(TEC):
                nc.tensor.matmul(out=pe1[0:1, :], lhsT=embs[:, j:j + 1],
                                 rhs=we_s[:, j, D:2 * D],
                                 start=(j == 0), stop=(j == TEC - 1))
            emb_part = embp.tile([1, 2 * D], bf16)
            nc.scalar.copy(out=emb_part[:, 0:D], in_=pe0[0:1, :])
            nc.scalar.copy(out=emb_part[:, D:2 * D], in_=pe1[0:1, :])

            u0 = psU.tile([P, 2, 2 * (Dh + 1)], f32, tag="u0")
            u1 = psU.tile([P, 2, 2 * (Dh + 1)], f32, tag="u1")
            usb = [embp.tile([64, H, Dh + 1], bf16, tag=f"usb{g}",
                             name=f"usb{g}")
                   for g in range(2)]

            deferred = []

            def q_post(i, qt):
                qs = work.tile([P, H, 1], f32, tag="qs")
                nc.vector.reduce_sum(
                    out=qs, in_=qt[:].rearrange("p (h d) -> p h d", h=H),
                    axis=mybir.AxisListType.X)
                nc.vector.reciprocal(out=rq_st[:, i, :, :], in_=qs)
                qTp = psT.tile([P, KC, P], bf16, tag="pT")
                for j in range(KC):
                    nc.tensor.transpose(qTp[:, j, :], qt[:, j * P:(j + 1) * P],
                                        ident)
                qT = qstash.tile([P, KC, P], bf16, tag="qT")
                q_tiles.append(qT)
                nc.vector.tensor_copy(out=qT, in_=qTp)

            def proj_tile(i, defer_q=False):
                # normalize on gpsimd (scalar tables untouched)
                xn = work.tile([P, D], bf16, tag="xn")
                nc.gpsimd.tensor_scalar(out=xn, in0=x_tiles[i],
                                        scalar1=rstd_a[:, i:i + 1],
                                        scalar2=nb_a[:, i:i + 1],
                                        op0=Alu.mult, op1=Alu.add)
                xTp = psT.tile([P, KC, P], bf16, tag="pT")
                for j in range(KC):
                    nc.tensor.transpose(xTp[:, j, :], xn[:, j * P:(j + 1) * P],
                                        ident)
                xT = work.tile([P, KC, P], f8, tag="xT")
                nc.scalar.copy(out=xT, in_=xTp)

                pq = psP.tile([P, D], f32, tag="pq", bufs=2)
                pk = psP.tile([P, D], f32, tag="pk")
                pv = psP.tile([P, D], f32, tag="pv")
                for m in range(2):
                    nc.tensor.matmul(out=pq, lhsT=xT[:, 2 * m:2 * m + 2, :],
                                     rhs=wq_s[:, 2 * m:2 * m + 2, :],
                                     start=(m == 0), stop=(m == 1),
                                     perf_mode=DR)
                for m in range(2):
                    nc.tensor.matmul(out=pk, lhsT=xT[:, 2 * m:2 * m + 2, :],
                                     rhs=wk_s[:, 2 * m:2 * m + 2, :],
                                     start=(m == 0), stop=(m == 1),
                                     perf_mode=DR)
                for m in range(2):
                    nc.tensor.matmul(out=pv, lhsT=xT[:, 2 * m:2 * m + 2, :],
                                     rhs=wv_s[:, 2 * m:2 * m + 2, :],
                                     start=(m == 0), stop=(m == 1),
                                     perf_mode=DR)
                if has_bq:
                    nc.vector.scalar_tensor_tensor(
                        out=pq, in0=pq, scalar=RWS, in1=bq_bc,
                        op0=Alu.mult, op1=Alu.add)
                if has_bk:
                    nc.vector.scalar_tensor_tensor(
                        out=pk, in0=pk, scalar=RWS, in1=bk_bc,
                        op0=Alu.mult, op1=Alu.add)
                if has_bv:
                    nc.vector.scalar_tensor_tensor(
                        out=pv, in0=pv, scalar=RWS, in1=bv_bc,
                        op0=Alu.mult, op1=Alu.add)
                qsc = 1.0 if has_bq else RWS
                ksc = 1.0 if has_bk else RWS
                vsc = 1.0 if has_bv else RWS

                # q: exp; per-head sums + transpose possibly deferred
                # past the final AllReduce to fill its latency window
                if defer_q:
                    qt = work.tile([P, D], bf16, tag="qtd", bufs=12)
                else:
                    qt = work.tile([P, D], bf16, tag="qt")
                nc.scalar.activation(out=qt, in_=pq, func=Act.Exp, scale=qsc)
                if defer_q:
                    deferred.append((i, qt))
                else:
                    q_post(i, qt)

                # k, v bf16; U einsum per head-pair
                et = work.tile([P, D], bf16, tag="et")
                nc.scalar.activation(out=et, in_=pk, func=Act.Exp, scale=ksc)
                va = work.tile([P, H, Dh + 1], bf16, tag="va")
                nc.vector.tensor_scalar(
                    out=va[:, :, 0:Dh],
                    in0=pv[:].rearrange("p (h d) -> p h d", h=H),
                    scalar1=mask_s[:, i:i + 1], scalar2=vsc,
                    op0=Alu.mult, op1=Alu.mult)
                nc.gpsimd.tensor_scalar_mul(out=va[:, :, Dh:Dh + 1],
                                            in0=ones8,
                                            scalar1=mask_s[:, i:i + 1])
                ig = i % NG
                for p2 in range(4):
                    u = u0 if p2 < 2 else u1
                    nc.tensor.matmul(
                        out=u[:, p2 % 2, :],
                        lhsT=et[:, p2 * P:(p2 + 1) * P],
                        rhs=va[:, 2 * p2:2 * p2 + 2, :].rearrange(
                            "p h f -> p (h f)"),
                        start=(ig == 0 and p2 % 2 == 0),
                        stop=(ig == NG - 1 and p2 % 2 == 1))

            def ship_u(g):
                u_sb = usb[g]
                for p2 in range(4):
                    u = u0 if p2 < 2 else u1
                    nc.scalar.copy(out=u_sb[:, 2 * p2, :],
                                   in_=u[0:64, p2 % 2, 0:Dh + 1])
                    nc.scalar.copy(out=u_sb[:, 2 * p2 + 1, :],
                                   in_=u[64:P, p2 % 2, Dh + 1:2 * Dh + 2])
                cc_in = cc_in_a if g == 0 else cc_in_b
                cc_out = cc_out_a if g == 0 else cc_out_b
                nc.sync.dma_start(
                    out=cc_in[0:CCU].rearrange("(p h f) -> p h f", p=64, h=H),
                    in_=u_sb)
                if g == 0:
                    nc.sync.dma_start(
                        out=cc_in[CCU:CCN].rearrange("(a f) -> a f", a=1),
                        in_=emb_part)
                nc.gpsimd.collective_compute(
                    "AllReduce", Alu.add, replica_groups=PAIRS,
                    ins=[cc_in[:]], outs=[cc_out[:]])

            # 8-tile stat groups; stats of group g+1 interleave with
            # projections of group g. U halves AllReduce at tiles 15/31.
            for g in range(3):
                for k in range(8):
                    stats_tile(8 * (g + 1) + k)
                    proj_tile(8 * g + k, defer_q=(8 * g + k >= 20))
                stats_batch(g + 1)
                if g == 1:
                    ship_u(0)
            for i in range(24, NT):
                proj_tile(i, defer_q=True)
            ship_u(1)
            for di, dqt in deferred:
                q_post(di, dqt)

        # ---- phase B prologue: attn state + stylization vectors ----
        with ExitStack() as ctxB:
            workB = ctxB.enter_context(tc.tile_pool(name="workB", bufs=3))
            psB = ctxB.enter_context(tc.tile_pool(name="psB", bufs=2, space="PSUM"))
            embB = ctxB.enter_context(tc.tile_pool(name="embB", bufs=1))

            # emb/stylization path first: it only needs the FIRST
            # AllReduce, so it executes inside the second AR's window
            emb_f = embB.tile([1, 2 * D], bf16)
            nc.sync.dma_start(
                out=emb_f, in_=cc_out_a[CCU:CCN].rearrange("(a f) -> a f", a=1))
            srow = embB.tile([1, D], f32)
            shrow = embB.tile([1, D], f32)
            if has_embb:
                nc.vector.tensor_add(out=srow, in0=emb_f[:, 0:D],
                                     in1=vec_s[:, 6, :])
                nc.vector.tensor_add(out=shrow, in0=emb_f[:, D:2 * D],
                                     in1=vec_s[:, 7, :])
            else:
                nc.vector.tensor_copy(out=srow, in_=emb_f[:, 0:D])
                nc.vector.tensor_copy(out=shrow, in_=emb_f[:, D:2 * D])
            t1 = embB.tile([1, D], f32)
            nc.vector.tensor_scalar_add(out=t1, in0=srow, scalar1=1.0)
            arow = embB.tile([1, D], f32)
            nc.vector.tensor_mul(out=arow, in0=t1, in1=vec_s[:, 4, :])
            crow = embB.tile([1, D], f32)
            nc.vector.tensor_mul(out=crow, in0=t1, in1=vec_s[:, 5, :])
            nc.vector.tensor_add(out=crow, in0=crow, in1=shrow)
            # transpose a,c rows to per-chunk columns [P, KC]
            acp = psB.tile([P, 2, KC], f32, tag="ac", bufs=1)
            for j in range(KC):
                nc.tensor.transpose(acp[:, 0, j:j + 1],
                                    arow[:, j * P:(j + 1) * P],
                                    one_f32)
                nc.tensor.transpose(acp[:, 1, j:j + 1],
                                    crow[:, j * P:(j + 1) * P],
                                    one_f32)
            a_col = embB.tile([P, KC], f32)
            nc.scalar.copy(out=a_col, in_=acp[:, 0, :])
            c_col = embB.tile([P, KC], f32)
            nc.scalar.copy(out=c_col, in_=acp[:, 1, :])

            # attn state (needs the second AllReduce)
            u_fa = embB.tile([P, H, Dh + 1], bf16)
            nc.sync.dma_start(
                out=u_fa[0:64], in_=cc_out_a[0:CCU].rearrange(
                    "(p h f) -> p h f", p=64, h=H))
            nc.sync.dma_start(
                out=u_fa[64:P], in_=cc_out_a[0:CCU].rearrange(
                    "(p h f) -> p h f", p=64, h=H))
            u_fb = embB.tile([P, H, Dh + 1], bf16)
            nc.sync.dma_start(
                out=u_fb[0:64], in_=cc_out_b[0:CCU].rearrange(
                    "(p h f) -> p h f", p=64, h=H))
            nc.sync.dma_start(
                out=u_fb[64:P], in_=cc_out_b[0:CCU].rearrange(
                    "(p h f) -> p h f", p=64, h=H))
            u_f = embB.tile([P, H, Dh + 1], f32)
            nc.vector.tensor_add(out=u_f, in0=u_fa, in1=u_fb)
            rs = embB.tile([P, H, 1], f32)
            nc.vector.reciprocal(out=rs, in_=u_f[:, :, Dh:Dh + 1])
            attn2 = embB.tile([P, KC, P], bf16)
            nc.gpsimd.memset(attn2, 0.0)
            for h in range(H):
                base = 64 * (h % 2)
                nc.vector.tensor_scalar_mul(
                    out=attn2[base:base + 64, h // 2, base:base + 64],
                    in0=u_f[base:base + 64, h, 0:Dh],
                    scalar1=rs[base:base + 64, h, :])

            mu2_a = stat.tile([P, NT], f32)
            var2_a = stat.tile([P, NT], f32)
            musq = stat.tile([P, NT], f32)

            def b1_tile(i):
                py = psB.tile([P, KC, P], f32, tag="py", bufs=2)
                for j in range(KC):
                    nc.tensor.matmul(out=py[:, j, :], lhsT=q_tiles[i][:, j, :],
                                     rhs=attn2[:, j, :], start=True, stop=True)
                ysb = ystash.tile([P, D], bf16, tag="ysb")
                y_tiles.append(ysb)
                # evac with broadcast 1/S; accum gives sum(y) for LN2 free
                nc.vector.scalar_tensor_tensor(
                    out=ysb[:].rearrange("p (h d) -> p h d", h=H),
                    in0=py[:].rearrange("p a b -> p (a b)").rearrange(
                        "p (h d) -> p h d", h=H),
                    scalar=1.0,
                    in1=rq_st[:, i, :, :].to_broadcast([P, H, Dh]),
                    op0=Alu.mult, op1=Alu.mult,
                    accum_out=s1_st[:, i:i + 1])
                # sum(y^2): alternate scalar ACT / vector STT to balance
                dumm = workB.tile([P, D], bf16, tag="dumm", bufs=2)
                if i % 2 == 0:
                    nc.scalar.activation(out=dumm, in_=ysb, func=Act.Square,
                                         accum_out=s2_st[:, i:i + 1])
                else:
                    nc.vector.scalar_tensor_tensor(
                        out=dumm, in0=ysb, scalar=1.0, in1=ysb,
                        op0=Alu.mult, op1=Alu.mult,
                        accum_out=s2_st[:, i:i + 1])

            def b_batch(g):
                sl = slice(g * NG, (g + 1) * NG)
                # var = E[y^2] - mu^2
                nc.vector.tensor_scalar_mul(out=mu2_a[:, sl], in0=s1_st[:, sl],
                                            scalar1=1.0 / D)
                nc.vector.tensor_scalar_mul(out=var2_a[:, sl],
                                            in0=s2_st[:, sl], scalar1=1.0 / D)
                nc.vector.tensor_mul(out=musq[:, sl], in0=mu2_a[:, sl],
                                     in1=mu2_a[:, sl])
                nc.vector.tensor_sub(out=var2_a[:, sl], in0=var2_a[:, sl],
                                     in1=musq[:, sl])
                nc.scalar.activation(out=sd2_a[:, sl], in_=var2_a[:, sl],
                                     func=Act.Ln, bias=eps_t)
                nc.scalar.activation(out=rstd2_a[:, sl], in_=sd2_a[:, sl],
                                     func=Act.Exp, scale=-0.5)
                nc.vector.tensor_mul(out=nb2_a[:, sl], in0=mu2_a[:, sl],
                                     in1=rstd2_a[:, sl])
                nc.vector.tensor_scalar_mul(out=nb2_a[:, sl],
                                            in0=nb2_a[:, sl], scalar1=-1.0)

            def b2_pair(i0):
                # LN2-normalize both tiles, PE-transpose into one PSUM bank,
                # then ONE ACT Silu per chunk covers both tiles' columns and
                # applies the stylization scale/bias (per-partition here)
                zs = []
                for t in range(2):
                    z = workB.tile([P, D], bf16, tag="z", bufs=4)
                    nc.gpsimd.tensor_scalar(out=z, in0=y_tiles[i0 + t],
                                            scalar1=rstd2_a[:, i0 + t:i0 + t + 1],
                                            scalar2=nb2_a[:, i0 + t:i0 + t + 1],
                                            op0=Alu.mult, op1=Alu.add)
                    zs.append(z)
                zTp = psB.tile([P, KC, 2, P], bf16, tag="pT")
                for t in range(2):
                    for j in range(KC):
                        nc.tensor.transpose(zTp[:, j, t, :],
                                            zs[t][:, j * P:(j + 1) * P], ident)
                hT2 = workB.tile([P, KC, 2, P], bf16, tag="hT")
                for j in range(KC):
                    nc.scalar.activation(
                        out=hT2[:, j, :, :].rearrange("p a b -> p (a b)"),
                        in_=zTp[:, j, :, :].rearrange("p a b -> p (a b)"),
                        func=Act.Silu, scale=a_col[:, j:j + 1],
                        bias=c_col[:, j:j + 1])
                for t in range(2):
                    i = i0 + t
                    po = psB.tile([P, D], f32, tag="po", bufs=3)
                    for j in range(KC):
                        nc.tensor.matmul(out=po, lhsT=hT2[:, j, t, :],
                                         rhs=wo_s[:, j, :],
                                         start=(j == 0), stop=(j == KC - 1))
                    xb = workB.tile([P, D], f32, tag="xb")
                    nc.sync.dma_start(out=xb, in_=x_in[i * P:(i + 1) * P, :])
                    osb = workB.tile([P, D], f32, tag="osb")
                    nc.vector.tensor_add(out=osb, in0=po, in1=xb)
                    if has_outb:
                        nc.vector.tensor_add(out=osb, in0=osb, in1=ob_bc)
                    nc.sync.dma_start(out=y_out[i * P:(i + 1) * P, :], in_=osb)

            # pipelined: B1 group 0 -> batch -> [B1 g1 || B2 g0] -> batch
            # -> B2 g1
            for i in range(NG):
                b1_tile(i)
            b_batch(0)
            for k in range(0, NG, 2):
                b1_tile(NG + k)
                b1_tile(NG + k + 1)
                b2_pair(k)
            b_batch(1)
            for i in range(NG, NT, 2):
                b2_pair(i)

    nc.compile()
    return nc


def _to_f8(a):
    return np.clip(a * WSCALE, -240.0, 240.0).astype(ml_dtypes.float8_e4m3fn)


def _prep(inputs, flags):
    bf = ml_dtypes.bfloat16
    x = np.asarray(inputs["x"], np.float32)
    emb = np.asarray(inputs["emb"], np.float32)
    src_mask = np.asarray(inputs["src_mask"], np.float32)
    gamma = np.asarray(inputs["gamma"], np.float32)
    beta = np.asarray(inputs["beta"], np.float32)
    gamma2 = np.asarray(inputs["gamma2"], np.float32)
    beta2 = np.asarray(inputs["beta2"], np.float32)
    emb_b = np.asarray(inputs["emb_b"], np.float32)
    out_b = np.asarray(inputs["out_b"], np.float32)

    def foldW(Wname):
        W = np.asarray(inputs[Wname], np.float32)
        return np.ascontiguousarray(
            _to_f8(gamma[:, None] * W).reshape(KC, P, D).transpose(1, 0, 2))

    wq, wk, wv = foldW("Wq"), foldW("Wk"), foldW("Wv")
    wo = np.ascontiguousarray(
        np.asarray(inputs["out_W"], np.float32).astype(bf).reshape(
            KC, P, D).transpose(1, 0, 2))
    bq_f = np.asarray(inputs["bq"], np.float32) + beta @ np.asarray(inputs["Wq"], np.float32)
    bk_f = np.asarray(inputs["bk"], np.float32) + beta @ np.asarray(inputs["Wk"], np.float32)
    bv_f = np.asarray(inputs["bv"], np.float32) + beta @ np.asarray(inputs["Wv"], np.float32)
    vecs = np.ascontiguousarray(np.stack(
        [bq_f, bk_f, bv_f, out_b, gamma2, beta2, emb_b[:D], emb_b[D:]]
    ).astype(np.float32).reshape(1, 8, D))
    emb_W = np.asarray(inputs["emb_W"], np.float32)
    we_halves = [
        np.ascontiguousarray(
            emb_W[t * TEH:(t + 1) * TEH].astype(bf).reshape(
                TEC, P, 2 * D).transpose(1, 0, 2))
        for t in range(2)]

    in_maps = []
    for c in range(NCORES):
        b, th = c // 2, c % 2
        sl = slice(th * TH, (th + 1) * TH)
        in_maps.append({
            "x": np.ascontiguousarray(x[b, sl]),
            "mask": np.ascontiguousarray(src_mask[b, sl, 0].reshape(NT, P).T),
            "embv": np.ascontiguousarray(
                emb[b, th * TEH:(th + 1) * TEH].reshape(TEC, P).T),
            "wq": wq, "wk": wk, "wv": wv, "wo": wo,
            "we": we_halves[th],
            "vecs": vecs,
        })
    return in_maps


def _flags(inputs):
    beta = np.asarray(inputs["beta"], np.float32)

    def nz(v):
        return bool(np.any(np.asarray(v) != 0))

    bq_f = np.asarray(inputs["bq"], np.float32) + beta @ np.asarray(inputs["Wq"], np.float32)
    bk_f = np.asarray(inputs["bk"], np.float32) + beta @ np.asarray(inputs["Wk"], np.float32)
    bv_f = np.asarray(inputs["bv"], np.float32) + beta @ np.asarray(inputs["Wv"], np.float32)
    return (nz(bq_f), nz(bk_f), nz(bv_f), nz(inputs["out_b"]), nz(inputs["emb_b"]))


def get_nc_and_inmaps(**inputs):
    flags = _flags(inputs)
    if flags not in _CACHE:
        _CACHE[flags] = _build(flags)
    return _CACHE[flags], _prep(inputs, flags)


def kernel(**inputs):
    from concourse.bass_utils import run_bass_kernel_spmd
    nc, in_maps = get_nc_and_inmaps(**inputs)
    res = run_bass_kernel_spmd(nc, in_maps, list(range(NCORES)))
    out = np.empty((B, T, D), np.float32)
    for c in range(NCORES):
        b, th = c // 2, c % 2
        out[b, th * TH:(th + 1) * TH] = res.results[c]["y"]
    return out
